# revision 42
# baseline (speedup 1.0000x reference)
"""GQA (B=1, S=2048, D=2048, 32 Q heads / 8 KV heads, head_dim=64, RoPE,
non-causal softmax) on 8 Trainium2 NeuronCores.

Sharding: tensor-parallel over heads. Core c owns Q heads 4c..4c+3 and KV head c.
Each core computes y_c = softmax(q_c k_c^T / 8) v_c @ Wo[:, c*256:(c+1)*256].T
(a full [S, D] partial); the host sums the 8 partials.

All matmul streams are bf16 (PSUM accumulation stays f32):
 - bf16 enables FWL so LDWEIGHTS hides behind the matmul stream, and halves
   input DMA so the PE starts much earlier.
 - score matmuls are K=64 row-tiled pairs (row_grp 0/64) streaming
   concurrently on the PE.
 - exp runs as [128,1024] two-bank ACT instructions; the attention loop is
   ACT-paced (~1.15us per k-tile), so Wo of the previous q-chunk and q-proj
   of the next q-chunk are emitted inside the kt loop as PE filler, which
   also keeps the PE HAM-warm (2.4 GHz).
 - softmax denominators ride as a ones-row in the v1 lhsT (pv row 64);
   1/l via SBUF-staged DVE reciprocal_approx_fast + matmul broadcast.
 - all cross-phase tensors (kTr chunks, q tiles, attention-out tiles) are
   small per-chunk tiles so Tile's dependency tracking never serializes a
   consumer on an unrelated producer.
"""

import numpy as np
import ml_dtypes

S = 2048
D = 2048
HD = 64
N_CORES = 8
ROPE_BASE = 10000.0

_cached = {}


def _build_program():
    import concourse.bass as bass
    import concourse.mybir as mybir
    import concourse.tile as tile
    from concourse import bacc

    BF16, F32 = mybir.dt.bfloat16, mybir.dt.float32
    EXP = mybir.ActivationFunctionType.Exp

    nc = bacc.Bacc("TRN2", target_bir_lowering=False, debug=False)

    xT = nc.dram_tensor("xT", [D, S], BF16, kind="ExternalInput").ap()
    wqp = nc.dram_tensor("wqp", [128, 4096], BF16, kind="ExternalInput").ap()
    wkvp = nc.dram_tensor("wkvp", [128, 2048], BF16, kind="ExternalInput").ap()
    wop = nc.dram_tensor("wop", [128, 4096], BF16, kind="ExternalInput").ap()
    cos2 = nc.dram_tensor("cos2", [128, S], F32, kind="ExternalInput").ap()
    sin2s = nc.dram_tensor("sin2s", [128, S], F32, kind="ExternalInput").ap()
    ones1 = nc.dram_tensor("ones1", [1, 64], BF16, kind="ExternalInput").ap()
    ident = nc.dram_tensor("ident", [64, 64], BF16, kind="ExternalInput").ap()
    y = nc.dram_tensor("y", [S, D], BF16, kind="ExternalOutput").ap()

    with tile.TileContext(nc) as tc:
        with tc.tile_pool(name="singles", bufs=1) as singles, \
             tc.tile_pool(name="rope", bufs=3) as rope, \
             tc.tile_pool(name="persist", bufs=1) as persist, \
             tc.tile_pool(name="vtcp", bufs=2) as vtcp, \
             tc.tile_pool(name="ptp", bufs=7) as ptp, \
             tc.tile_pool(name="smp", bufs=4) as smp, \
             tc.tile_pool(name="qtp", bufs=4) as qtp, \
             tc.tile_pool(name="otq", bufs=4) as otq, \
             tc.tile_pool(name="ysbp", bufs=4) as ysbp, \
             tc.tile_pool(name="big", bufs=2, space="PSUM") as bigp, \
             tc.tile_pool(name="mmp", bufs=2, space="PSUM") as mmp, \
             tc.tile_pool(name="pvp", bufs=2, space="PSUM") as pvp, \
             nc.allow_low_precision(reason="bf16 matmul paths are intended"):

            # ---- input DMAs: x tiles on the sync queue (needed first),
            # ---- statics on the scalar queue (ACT idle at startup).
            wkv_sb = singles.tile([128, 2048], BF16, tag="wkv")
            nc.scalar.dma_start(out=wkv_sb, in_=wkvp)
            wq_sb = singles.tile([128, 4096], BF16, tag="wq")
            nc.scalar.dma_start(out=wq_sb, in_=wqp)
            ones_sb = singles.tile([1, 64], BF16, tag="ones1")
            nc.sync.dma_start(out=ones_sb, in_=ones1)
            ident_sb = singles.tile([64, 64], BF16, tag="ident")
            nc.sync.dma_start(out=ident_sb, in_=ident)
            x_sb = [None] * 16
            qassign = {nc.sync: [0, 3, 6, 9, 12, 14],
                       nc.gpsimd: [1, 4, 7, 10, 13, 15],
                       nc.scalar: [2, 5, 8, 11]}
            for q, ks in qassign.items():
                for k in ks:
                    t = singles.tile([128, S], BF16, tag=f"x{k}", name=f"x{k}")
                    q.dma_start(out=t, in_=xT[k * 128:(k + 1) * 128, :])
                    x_sb[k] = t
            cos_sb = singles.tile([128, S], F32, tag="cos")
            nc.scalar.dma_start(out=cos_sb, in_=cos2)
            sin_sb = singles.tile([128, S], F32, tag="sin")
            nc.scalar.dma_start(out=sin_sb, in_=sin2s)
            wo_sb = singles.tile([128, 4096], BF16, tag="wo")
            nc.scalar.dma_start(out=wo_sb, in_=wop)

            # per-chunk k tiles (rows 64:128 duplicate rows 0:64)
            kTr = [persist.tile([128, 512], BF16, tag=f"kTr{c}", name=f"kTr{c}")
                   for c in range(4)]
            v1 = [singles.tile([128, 65], BF16, tag=f"v1_{kt}", name=f"v1_{kt}")
                  for kt in range(16)]
            # per-(qc, blk) RoPE'd q tiles; per-(qc, hp) attention outputs
            qt = {}
            ot = {}

            def rope_q(acc, dst, cols, fast=False):
                """dst = acc*cos + rotate_half(acc)*sin for a [128,512] block."""
                t1 = rope.tile([128, 512], F32, tag="t1")
                t2 = rope.tile([128, 512], F32, tag="t2")
                nc.vector.tensor_mul(t1, acc, cos_sb[:, cols])
                nc.vector.tensor_mul(t2[0:32], acc[32:64], sin_sb[0:32, cols])
                nc.vector.tensor_mul(t2[32:64], acc[0:32], sin_sb[32:64, cols])
                nc.vector.tensor_mul(t2[64:96], acc[96:128], sin_sb[64:96, cols])
                nc.vector.tensor_mul(t2[96:128], acc[64:96], sin_sb[96:128, cols])
                (nc.vector if fast else nc.gpsimd).tensor_add(dst, t1, t2)

            # ---- startup: k-major over q-chunk0 blocks and kv chunks 0/1
            # ---- so the PE eats x tiles as they arrive; kv chunks 2/3 run
            # ---- later as attention fillers.
            def krope_chunk(ch, acc, fast=False):
                chs = slice(ch * 512, (ch + 1) * 512)
                t1 = rope.tile([64, 512], F32, tag="kt1", name=f"kt1_{ch}")
                t2 = rope.tile([64, 512], F32, tag="kt2", name=f"kt2_{ch}")
                nc.vector.tensor_mul(t1, acc[0:64], cos_sb[0:64, chs])
                nc.vector.tensor_mul(t2[0:32], acc[32:64], sin_sb[0:32, chs])
                nc.vector.tensor_mul(t2[32:64], acc[0:32], sin_sb[32:64, chs])
                (nc.vector if fast else nc.gpsimd).tensor_add(kTr[ch][0:64, :], t1, t2)
                nc.vector.tensor_copy(kTr[ch][64:128, :], kTr[ch][0:64, :])
                vtc = vtcp.tile([64, 512], BF16, tag="vtc", name=f"vtc{ch}")
                nc.vector.tensor_copy(vtc, acc[64:128])
                for b in range(4):
                    kt = ch * 4 + b
                    tp = mmp.tile([128, 64], BF16, tag="mm", name=f"tp{kt}")
                    nc.tensor.transpose(tp, vtc[:, b * 128:(b + 1) * 128], ident_sb)
                    nc.vector.tensor_copy(v1[kt][:, 0:64], tp)
                    nc.gpsimd.memset(v1[kt][:, 64:65], 1.0)

            kvacc = bigp.tile([128, 1024], F32, tag="big", name="kvacc01")
            qacc0 = [mmp.tile([128, 512], F32, tag="mm", name=f"qacc0_{i}")
                     for i in range(2)]
            for k in range(16):
                lkv = wkv_sb[:, k * 128:(k + 1) * 128]
                for ch in range(2):
                    nc.tensor.matmul(kvacc[:, ch * 512:(ch + 1) * 512],
                                     lkv, x_sb[k][:, ch * 512:(ch + 1) * 512],
                                     start=(k == 0), stop=(k == 15))
                for blk in range(2):
                    lq = wq_sb[:, k * 256 + blk * 128: k * 256 + blk * 128 + 128]
                    nc.tensor.matmul(qacc0[blk], lq, x_sb[k][:, 0:512],
                                     start=(k == 0), stop=(k == 15))

            qt[(0, 0)] = qtp.tile([128, 512], BF16, tag="qt", name="qt_0_0")
            rope_q(qacc0[0], qt[(0, 0)], slice(0, 512), fast=True)
            krope_chunk(0, kvacc[:, 0:512], fast=True)
            qt[(0, 1)] = qtp.tile([128, 512], BF16, tag="qt", name="qt_0_1")
            rope_q(qacc0[1], qt[(0, 1)], slice(0, 512))
            krope_chunk(1, kvacc[:, 512:1024])

            # ---- attention; Wo(qc-1) and q-proj(qc+1) interleave as filler.
            def emit_wo_unit(wqc, st, mc, anyeng=False):
                ss = slice(wqc * 512 + st * 128, wqc * 512 + (st + 1) * 128)
                ms = slice(mc * 512, (mc + 1) * 512)
                yac = mmp.tile([128, 512], F32, tag="mm", name=f"y_{wqc}_{st}_{mc}")
                nc.tensor.matmul(yac, ot[(wqc, 0)][:, st * 128:(st + 1) * 128],
                                 wo_sb[:, mc * 512:(mc + 1) * 512],
                                 start=True, stop=False)
                nc.tensor.matmul(yac, ot[(wqc, 1)][:, st * 128:(st + 1) * 128],
                                 wo_sb[:, 2048 + mc * 512:2048 + (mc + 1) * 512],
                                 start=False, stop=True)
                ysb = ysbp.tile([128, 512], BF16, tag="ysb")
                if anyeng:
                    nc.any.tensor_copy(ysb, yac)
                else:
                    nc.vector.tensor_copy(ysb, yac)
                nc.gpsimd.dma_start(out=y[ss, ms], in_=ysb)

            pending_drain = []
            for qc in range(4):
                # schedule: q-proj runs as two compact 16-MM bursts (back-to-
                # back keeps HAM warm and the psum acc lifetime short); wo
                # units spread across the remaining slots; slots 0/1 are
                # reserved for the previous head-pair's deferred drain.
                sched = [[] for _ in range(32)]
                if qc == 0:
                    for k in range(16):
                        sched[k // 4].append(("kv", 2, k))
                        sched[4 + k // 4].append(("kv", 3, k))
                    for k in range(16):
                        sched[8].append(("qp", 1, 0, k))
                        sched[20].append(("qp", 1, 1, k))
                else:
                    wo_units = [("wo", qc - 1, st, mc) for st in range(4)
                                for mc in range(4)]
                    if qc < 3:
                        for k in range(16):
                            sched[4].append(("qp", qc + 1, 0, k))
                            sched[20].append(("qp", qc + 1, 1, k))
                        wo_slots = [2, 3, 7, 8, 9, 11, 13, 15, 17, 18, 22, 24,
                                    25, 26, 27, 28]
                    else:
                        wo_slots = [2, 3, 5, 7, 9, 11, 13, 15, 17, 19, 21, 22,
                                    23, 24, 25, 26]
                    for s, u in zip(wo_slots, wo_units):
                        sched[s].append(u)
                qp_state = {}
                kv_state = {}

                def emit_fillers(slot):
                    for f in sched[slot]:
                        if f[0] == "wo":
                            emit_wo_unit(f[1], f[2], f[3])
                        elif f[0] == "kv":
                            _, ch, k = f
                            if ch not in kv_state:
                                kv_state[ch] = mmp.tile(
                                    [128, 512], F32, tag="mm",
                                    name=f"kvacc{ch}")
                            lkv = wkv_sb[:, k * 128:(k + 1) * 128]
                            nc.tensor.matmul(kv_state[ch], lkv,
                                             x_sb[k][:, ch * 512:(ch + 1) * 512],
                                             start=(k == 0), stop=(k == 15))
                            if k == 15:
                                krope_chunk(ch, kv_state.pop(ch))
                        else:
                            _, nqc, blk, k = f
                            if blk not in qp_state:
                                qp_state[blk] = mmp.tile(
                                    [128, 512], F32, tag="mm",
                                    name=f"qacc_{nqc}_{blk}")
                            lq = wq_sb[:, k * 256 + blk * 128: k * 256 + blk * 128 + 128]
                            nc.tensor.matmul(qp_state[blk], lq,
                                             x_sb[k][:, nqc * 512:(nqc + 1) * 512],
                                             start=(k == 0), stop=(k == 15))
                            if k == 15:
                                qt[(nqc, blk)] = qtp.tile(
                                    [128, 512], BF16, tag="qt",
                                    name=f"qt_{nqc}_{blk}")
                                rope_q(qp_state.pop(blk), qt[(nqc, blk)],
                                       slice(nqc * 512, (nqc + 1) * 512))

                for hp in range(2):
                    qsrc = qt[(qc, hp)]
                    dst = ot[(qc, hp)] = otq.tile([128, 512], BF16, tag="ot",
                                                  name=f"ot_{qc}_{hp}")
                    pvA = pvp.tile([65, 512], F32, tag="pv", name=f"pvA_{qc}_{hp}")
                    pvB = pvp.tile([65, 512], F32, tag="pv", name=f"pvB_{qc}_{hp}")
                    pend_pv = []
                    for kt in range(16):
                        bi = kt % 4
                        sps = bigp.tile([128, 1024], F32, tag="big",
                                        name=f"sps_{qc}_{hp}_{kt}")
                        nc.tensor.matmul(sps[:, 0:512],
                                         kTr[kt // 4][0:64, bi * 128:(bi + 1) * 128],
                                         qsrc[0:64, :], start=True, stop=True)
                        nc.tensor.matmul(sps[:, 512:1024],
                                         kTr[kt // 4][64:128, bi * 128:(bi + 1) * 128],
                                         qsrc[64:128, :], start=True, stop=True)
                        pt = ptp.tile([128, 1024], BF16, tag="pt",
                                      name=f"pt_{qc}_{hp}_{kt}")
                        nc.scalar.activation(pt, sps, EXP, scale=0.125)
                        # pv lags TWO kts: by the time the in-order PE queue
                        # reaches a pv matmul, its exp (issued ~2.2us earlier on
                        # the saturated ACT queue) has completed, so the queue
                        # head never stalls and fillers fit in the slack
                        if len(pend_pv) >= 3:
                            pkt, ppt = pend_pv.pop(0)
                            nc.tensor.matmul(pvA, v1[pkt], ppt[:, 0:512],
                                             start=(pkt == 0), stop=False)
                            nc.tensor.matmul(pvB, v1[pkt], ppt[:, 512:1024],
                                             start=(pkt == 0), stop=False)
                        pend_pv.append((kt, pt))
                        # PE half of the previous head-pair's softmax drain
                        if kt < 2 and pending_drain:
                            pending_drain.pop(0)()
                        emit_fillers(hp * 16 + kt)
                    for pkt, ppt in pend_pv:
                        nc.tensor.matmul(pvA, v1[pkt], ppt[:, 0:512],
                                         start=(pkt == 0), stop=(pkt == 15))
                        nc.tensor.matmul(pvB, v1[pkt], ppt[:, 512:1024],
                                         start=(pkt == 0), stop=(pkt == 15))
                    for step in pending_drain:
                        step()
                    pending_drain = []
                    for sub, pv in ((0, pvA), (1, pvB)):
                        lsb = smp.tile([1, 512], F32, tag="lsb",
                                       name=f"lsb_{qc}_{hp}_{sub}")
                        rc = smp.tile([1, 512], F32, tag="rc",
                                      name=f"rc_{qc}_{hp}_{sub}")
                        rcb = smp.tile([1, 512], BF16, tag="rcb",
                                       name=f"rcb_{qc}_{hp}_{sub}")
                        nc.vector.tensor_copy(lsb, pv[64:65, :])
                        nc.vector.reciprocal_approx_fast(out=rc, in_=lsb)
                        nc.vector.tensor_copy(rcb, rc)

                        def drain_pe(pv=pv, rcb=rcb, dst=dst, sub=sub,
                                     nm=f"{qc}_{hp}_{sub}"):
                            bps = mmp.tile([64, 512], F32, tag="mm",
                                           name=f"bps_{nm}")
                            nc.tensor.matmul(bps, ones_sb, rcb,
                                             start=True, stop=True)
                            bsb = smp.tile([64, 512], F32, tag="bsb",
                                           name=f"bsb_{nm}")
                            nc.vector.tensor_copy(bsb, bps)
                            nc.vector.tensor_mul(dst[sub * 64:(sub + 1) * 64, :],
                                                 pv[0:64, :], bsb)
                        pending_drain.append(drain_pe)
            for step in pending_drain:
                step()
            pending_drain = []
            # tail: Wo for the last chunk (ACT is idle, let nc.any use it)
            for st in range(4):
                for mc in range(4):
                    emit_wo_unit(3, st, mc, anyeng=True)

    nc.compile()
    return nc


def _host_prep(x, Wq, Wk, Wv, Wo):
    """Build per-core input maps (host-side numpy, untimed)."""
    bf16 = ml_dtypes.bfloat16
    x2 = np.ascontiguousarray(x.reshape(S, D), dtype=np.float32)
    xT = np.ascontiguousarray(x2.T).astype(bf16)

    inv = 1.0 / (ROPE_BASE ** (np.arange(0, HD, 2, dtype=np.float32) / HD))
    t = np.arange(S, dtype=np.float32)
    ang = np.einsum("i,j->ij", t, inv)              # [S, 32]
    emb = np.concatenate([ang, ang], axis=-1)       # [S, 64]
    cosT = np.ascontiguousarray(np.cos(emb).T.astype(np.float32))   # [64, S]
    sinT = np.ascontiguousarray(np.sin(emb).T.astype(np.float32))
    sinTs = sinT.copy()
    sinTs[0:32] *= -1.0
    cos2 = np.ascontiguousarray(np.concatenate([cosT, cosT], axis=0))
    sin2s = np.ascontiguousarray(np.concatenate([sinTs, sinTs], axis=0))

    ones1 = np.ones((1, 64), dtype=np.float32).astype(bf16)
    identm = np.eye(64, dtype=np.float32).astype(bf16)

    in_maps = []
    for c in range(N_CORES):
        osl = slice(c * 256, (c + 1) * 256)
        ksl = slice(c * 64, (c + 1) * 64)
        wqt = np.ascontiguousarray(Wq[osl, :].T.astype(np.float32))          # [D, 256]
        wqp = np.ascontiguousarray(
            wqt.reshape(16, 128, 256).transpose(1, 0, 2).reshape(128, 4096)
        ).astype(bf16)
        wkvt = np.ascontiguousarray(
            np.concatenate([Wk[ksl, :], Wv[ksl, :]], axis=0).T.astype(np.float32))  # [D, 128]
        wkvp = np.ascontiguousarray(
            wkvt.reshape(16, 128, 128).transpose(1, 0, 2).reshape(128, 2048)
        ).astype(bf16)
        wot = np.ascontiguousarray(Wo[:, osl].T.astype(np.float32))          # [256, D]
        wop = np.ascontiguousarray(
            wot.reshape(2, 128, 2048).transpose(1, 0, 2).reshape(128, 4096)
        ).astype(bf16)
        in_maps.append({
            "xT": xT, "wqp": wqp, "wkvp": wkvp, "wop": wop,
            "cos2": cos2, "sin2s": sin2s,
            "ones1": ones1, "ident": identm,
        })
    return in_maps


def kernel(x, Wq, Wk, Wv, Wo, _trace=False):
    from concourse.bass_utils import run_bass_kernel_spmd

    x = np.asarray(x, dtype=np.float32)
    Wq = np.asarray(Wq, dtype=np.float32)
    Wk = np.asarray(Wk, dtype=np.float32)
    Wv = np.asarray(Wv, dtype=np.float32)
    Wo = np.asarray(Wo, dtype=np.float32)

    if "nc" not in _cached:
        _cached["nc"] = _build_program()
    nc = _cached["nc"]

    in_maps = _host_prep(x, Wq, Wk, Wv, Wo)
    res = run_bass_kernel_spmd(nc, in_maps, core_ids=list(range(N_CORES)),
                               trace=_trace)
    out = np.zeros((S, D), dtype=np.float64)
    for r in res.results:
        out += r["y"].astype(np.float64)
    _cached["last_results"] = res
    return out.astype(np.float32).reshape(1, S, D)
